# revision 1
# baseline (speedup 1.0000x reference)
"""GATv2 layer on 8 Trainium2 NeuronCores (Bass/Tile).

Strategy (edge-parallel by target-node slice, no collectives):
  - Node n belongs to core n // 12500. Each core computes output rows for its
    12500 nodes, handling exactly the edges whose tgt is in its slice.
  - Phase H: h_r = features @ w_r for ALL nodes (each core, 782 tiles) into an
    HBM table [100000, 64]; h_l = features @ w_l for the core's own slice into
    [12544, 64].
  - Phase E: edges sorted by (src chunk of 25000, tgt). Greedy "windows" of
    512 slots covering <=128 consecutive tgt nodes (whole-node assignment).
    Per 4096-slot batch: dma_gather x_j rows (1 descriptor/edge, int16 local
    idx per chunk); x_i is NOT gathered - it is expanded on-chip with a
    one-hot SelT matmul against the window's h_l rows (fetched by a small
    per-batch row gather). scores -> exp (ACT) -> messages (DVE); per-window
    aggregation via one-hot Sel matmuls accumulating in PSUM; window results
    scatter-added (dma_scatter_add) into per-chunk HBM accumulators.
  - Phase M: merge the 4 accumulators, divide by denominators, add bias.

Numerics: softmax computed without the max-shift (scores are O(+-15) so
exp() is safe in f32); this matches the reference to ~1e-6 rel.
"""

import sys
import types

sys.path.insert(0, "/opt/trn_rl_repo")

import numpy as np

N, E, F_IN, H, F_OUT = 100000, 1600000, 128, 4, 16
HF = H * F_OUT            # 64
NEG_SLOPE = 0.2
NCORES = 8
NLOC = N // NCORES        # 12500
NCHUNK = 4
CHUNK = 32768             # int16-addressable rows per x_j chunk
WSLOTS = 512              # slots per window
WTILES = WSLOTS // 128    # 4 tiles per window
BWIN = 8                  # windows per batch
BSLOTS = BWIN * WSLOTS    # 4096 slots per batch
BTILES = BSLOTS // 128    # 32 tiles per batch
NLOCP = 12544             # 98*128, padded local rows
ACCROWS = NLOCP + 128     # + scratch rows for padded window tails


# ----------------------------------------------------------------- host prep
def _prep_core(src, tgt, core):
    """Slot/window layout for one core. Returns dict of per-run arrays."""
    n0 = core * NLOC
    m = (tgt >= n0) & (tgt < n0 + NLOC)
    s, t = src[m], tgt[m] - n0
    chunk = s // CHUNK
    order = np.lexsort((t, chunk))
    s, t, chunk = s[order], t[order], chunk[order]
    runs = []
    bounds = np.searchsorted(chunk, np.arange(NCHUNK + 1))
    for r in range(NCHUNK):
        lo, hi = bounds[r], bounds[r + 1]
        ts, ss = t[lo:hi], s[lo:hi] - r * CHUNK
        # whole-node greedy windows: <=512 slots, <=128 node span
        nodes, counts = np.unique(ts, return_counts=True)
        cum = np.concatenate([[0], np.cumsum(counts)])
        wins = []  # (base, edge_lo, edge_hi)
        i = 0
        while i < len(nodes):
            base = nodes[i]
            j_span = np.searchsorted(nodes, base + 128, side="left")
            j_slot = np.searchsorted(cum, cum[i] + WSLOTS, side="right") - 1
            j = max(i + 1, min(j_span, j_slot))
            wins.append((base, cum[i], cum[j]))
            i = j
        # safe span: rows [base_w, base_w + safe_w) are owned exclusively by
        # window w (next window starts at its own base) -> no scatter races
        spans = []
        for w, (base, _, _) in enumerate(wins):
            nxt = wins[w + 1][0] if w + 1 < len(wins) else base + 128
            spans.append(min(nxt - base, 128))
        runs.append({"wins": wins, "spans": spans, "s": ss, "t": ts})
    return runs


def _assemble(runs_all, W_r):
    """Build per-core flat arrays given static per-run window counts W_r."""
    out = []
    for core in range(NCORES):
        xj_idx, tsel, pidx, sidx = [], [], [], []
        for r in range(NCHUNK):
            run = runs_all[core][r]
            wins = run["wins"]
            ss, ts = run["s"], run["t"]
            spans = run["spans"]
            scratch = (NLOCP + np.arange(128)).astype(np.int16)
            for w in range(W_r[r]):
                if w < len(wins):
                    base, elo, ehi = wins[w]
                    ne = ehi - elo
                    safe = spans[w]
                else:
                    base, elo, ne, safe = 0, 0, 0, 0  # pad window
                xw = np.zeros(WSLOTS, np.int16)
                tw = np.full(WSLOTS, -1, np.int8)
                if ne:
                    xw[:ne] = ss[elo:elo + ne]
                    tw[:ne] = (ts[elo:elo + ne] - base).astype(np.int8)
                xj_idx.append(xw)
                tsel.append(tw)
                # h_l row gather for this window (clamped to valid rows)
                pr = np.minimum(base + np.arange(128), NLOC - 1).astype(np.int16)
                pidx.append(pr)
                # scatter rows: exclusively-owned rows for p < safe, else
                # scratch (zero adds; races on scratch are zero+zero = safe)
                p = np.arange(128)
                sr = np.where((p < safe) & (base + p < NLOC),
                              base + p, scratch).astype(np.int16)
                sidx.append(sr)
        out.append({
            "xj_idx": np.concatenate(xj_idx),       # [S_total] int16
            "tsel": np.concatenate(tsel),           # [S_total] int8
            "pidx": np.concatenate(pidx),           # [W_total*128] int16
            "sidx": np.concatenate(sidx),           # [W_total*128] int16
        })
    return out


def _wrap16(ix):
    """int16 index layout for dma_gather/scatter: i -> (i%16, i//16), x8."""
    w2 = ix.reshape(-1, 16).T
    return np.tile(w2, (8, 1)).copy()


def prep(edge_index):
    src = np.asarray(edge_index[0], dtype=np.int64).astype(np.int32)
    tgt = np.asarray(edge_index[1], dtype=np.int64).astype(np.int32)
    runs_all = [_prep_core(src, tgt, c) for c in range(NCORES)]
    W_r = []
    for r in range(NCHUNK):
        w = max(len(runs_all[c][r]["wins"]) for c in range(NCORES))
        W_r.append(-(-w // (2 * BWIN)) * (2 * BWIN))  # round to gather-pair
    cores = _assemble(runs_all, W_r)
    return W_r, cores


DEBUG = False


# ------------------------------------------------------------- device kernel
def build(W_r):
    import concourse.bass as bass
    import concourse.mybir as mybir
    import concourse.tile as tile
    from concourse import bacc

    dt = mybir.dt
    W_total = sum(W_r)
    S_total = W_total * WSLOTS
    NT_total = S_total // 128

    nc = bacc.Bacc("TRN2", target_bir_lowering=False, num_swdge_queues=4,
                   dynamic_dma_scratch_size=32768)

    # ---- inputs (per-core data, same shapes everywhere)
    featT = nc.dram_tensor("featT", [128, N], dt.float32, kind="ExternalInput")
    featT_loc = nc.dram_tensor("featT_loc", [128, NLOCP], dt.float32,
                               kind="ExternalInput")
    w_lr = nc.dram_tensor("w_lr", [128, 2 * HF], dt.float32,
                          kind="ExternalInput")  # [:,:64]=w_l [:,64:]=w_r
    att_b = nc.dram_tensor("att_b", [128, BSLOTS // 128 * HF], dt.float32,
                           kind="ExternalInput")  # att row tiled 32x
    bias_b = nc.dram_tensor("bias_b", [128, HF], dt.float32,
                            kind="ExternalInput")
    xj_idx = nc.dram_tensor("xj_idx", [128, S_total // 16], dt.int16,
                            kind="ExternalInput")
    tsel_rep = nc.dram_tensor("tsel_rep", [128, S_total], dt.int8,
                              kind="ExternalInput")
    tsel_col = nc.dram_tensor("tsel_col", [128, NT_total], dt.int8,
                              kind="ExternalInput")
    pidx = nc.dram_tensor("pidx", [128, W_total * 128 // 16], dt.int16,
                          kind="ExternalInput")
    sidx = nc.dram_tensor("sidx", [128, W_total * 128 // 16], dt.int16,
                          kind="ExternalInput")

    # ---- HBM scratch
    h_rc = [nc.dram_tensor(f"h_r{r}", [min(CHUNK, N - r * CHUNK), HF],
                           dt.float32, kind="Internal")
            for r in range(NCHUNK)]
    h_l = nc.dram_tensor("h_l", [NLOCP, HF], dt.float32, kind="Internal")
    accs = [nc.dram_tensor(f"acc{r}", [ACCROWS, 128], dt.float32,
                           kind="ExternalInput") for r in range(NCHUNK)]
    out = nc.dram_tensor("out", [NLOCP, HF], dt.float32, kind="ExternalOutput")
    dbg = {}
    if DEBUG:
        for r in range(NCHUNK):
            dbg[f"acc{r}"] = nc.dram_tensor(f"dbg_acc{r}", [ACCROWS, 128],
                                            dt.float32, kind="ExternalOutput")

    AF = mybir.ActivationFunctionType
    OP = mybir.AluOpType

    with tile.TileContext(nc) as tc:
        with (
            tc.tile_pool(name="hph", bufs=3) as hph,
            tc.tile_pool(name="hps", bufs=3) as hps,
            tc.tile_pool(name="hpp", bufs=2, space="PSUM") as hpp,
            tc.tile_pool(name="cst", bufs=1) as cst,
            tc.tile_pool(name="cst2", bufs=1) as cst2,
            tc.tile_pool(name="idxp", bufs=2) as idxp,
            tc.tile_pool(name="gp", bufs=2) as gp,
            tc.tile_pool(name="hlw", bufs=2) as hlw,
            tc.tile_pool(name="selp", bufs=2) as selp,
            tc.tile_pool(name="zp", bufs=2) as zp,
            tc.tile_pool(name="msgp", bufs=2) as msgp,
            tc.tile_pool(name="scp", bufs=2) as scp,
            tc.tile_pool(name="stg", bufs=2) as stg,
            tc.tile_pool(name="pz", bufs=3, space="PSUM") as pz,
            tc.tile_pool(name="pw", bufs=2, space="PSUM") as pw,
        ):
            # ---------------- phase H: h tables --------------------------
            # h_l first, then h_r chunk-by-chunk in the order phase E
            # consumes them, so E's first gathers can start early.
            wt = cst.tile([128, 2 * HF], dt.float32)
            nc.scalar.dma_start(out=wt[:], in_=w_lr[:])
            for g in range(NLOCP // 128):       # h_l for own slice
                ft = hph.tile([128, 128], dt.float32)
                nc.scalar.dma_start(out=ft[:], in_=featT_loc[:, 128 * g:128 * (g + 1)])
                hp = hpp.tile([128, HF], dt.float32, space="PSUM")
                nc.tensor.matmul(out=hp[:], lhsT=ft[:], rhs=wt[:, :HF],
                                 start=True, stop=True)
                hs = hps.tile([128, HF], dt.float32)
                nc.vector.tensor_copy(out=hs[:], in_=hp[:])
                nc.scalar.dma_start(out=h_l[128 * g:128 * (g + 1), :], in_=hs[:])
            # constants
            att_t = cst2.tile([128, BTILES * HF], dt.float32)
            nc.sync.dma_start(out=att_t[:], in_=att_b[:])
            iota_i = cst2.tile([128, 128], dt.int32)
            nc.gpsimd.iota(iota_i[:], pattern=[[1, 128]], base=0,
                           channel_multiplier=0)
            iota_row = cst2.tile([128, 128], dt.int8)
            nc.vector.tensor_copy(out=iota_row[:], in_=iota_i[:])
            iota_c = cst2.tile([128, 1], dt.int32)
            nc.gpsimd.iota(iota_c[:], pattern=[[0, 1]], base=0,
                           channel_multiplier=1)
            iota_col = cst2.tile([128, 1], dt.float32)
            nc.vector.tensor_copy(out=iota_col[:], in_=iota_c[:])

            # ---------------- phase E: edges -----------------------------
            # Gathers/scatter span a PAIR of compute batches (2x4096 slots)
            # to amortize SWDGE descriptor-gen fixed costs.
            qi = 0  # SWDGE queue rotation
            PW = 2 * BWIN          # windows per pair
            PS = 2 * BSLOTS        # slots per pair
            for r in range(NCHUNK):
                w_off = sum(W_r[:r])   # global window offset
                csz = min(CHUNK, N - r * CHUNK)
                for g in range(-(-csz // 128)):
                    n0g = 128 * g
                    n1g = min(128 * (g + 1), csz)
                    w = n1g - n0g
                    ft = hph.tile([128, 128], dt.float32)
                    nc.scalar.dma_start(out=ft[:, :w],
                                      in_=featT[:, r * CHUNK + n0g:r * CHUNK + n1g])
                    hp = hpp.tile([128, HF], dt.float32, space="PSUM")
                    nc.tensor.matmul(out=hp[:w, :], lhsT=ft[:, :w],
                                     rhs=wt[:, HF:], start=True, stop=True)
                    hs = hps.tile([128, HF], dt.float32)
                    nc.vector.tensor_copy(out=hs[:w, :], in_=hp[:w, :])
                    nc.scalar.dma_start(out=h_rc[r][n0g:n1g, :], in_=hs[:w, :])

                np_pairs = W_r[r] // PW
                for b in range(np_pairs):
                    wg = w_off + b * PW            # first window, global
                    s0 = wg * WSLOTS               # first slot, global
                    # --- loads
                    it = idxp.tile([128, PS // 16], dt.int16, tag="it")
                    nc.sync.dma_start(out=it[:], in_=xj_idx[:, s0 // 16:(s0 + PS) // 16])
                    pit = idxp.tile([128, PW * 8], dt.int16, tag="pit")
                    nc.sync.dma_start(out=pit[:], in_=pidx[:, wg * 8:(wg + PW) * 8])
                    sit = idxp.tile([128, PW * 8], dt.int16, tag="sit")
                    nc.sync.dma_start(out=sit[:], in_=sidx[:, wg * 8:(wg + PW) * 8])
                    # --- gathers (one per pair)
                    xj = gp.tile([128, 2 * BTILES, HF], dt.float32)
                    nc.gpsimd.dma_gather(xj[:], h_rc[r][:], it[:],
                                         PS, PS, HF,
                                         single_packet=False, queue_num=qi % 4)
                    qi += 1
                    hper = hlw.tile([128, PW, HF], dt.float32)
                    nc.gpsimd.dma_gather(hper[:], h_l[:], pit[:],
                                         PW * 128, PW * 128, HF,
                                         single_packet=False, queue_num=qi % 4)
                    qi += 1
                    stag = stg.tile([128, PW, 128], dt.float32)
                    for half in range(2):
                        hs0 = s0 + half * BSLOTS
                        ht0 = hs0 // 128
                        tsr = idxp.tile([128, BSLOTS], dt.int8, tag="tsr")
                        nc.sync.dma_start(out=tsr[:],
                                          in_=tsel_rep[:, hs0:hs0 + BSLOTS])
                        tsc = idxp.tile([128, BTILES], dt.int8, tag="tsc")
                        nc.sync.dma_start(out=tsc[:],
                                          in_=tsel_col[:, ht0:ht0 + BTILES])
                        # --- Sel / SelT builds (one DVE op each)
                        selT = selp.tile([128, BSLOTS], dt.float32, tag="selT")
                        nc.vector.tensor_scalar(out=selT[:], in0=tsr[:],
                                                scalar1=iota_col[:], scalar2=None,
                                                op0=OP.is_equal)
                        sel = selp.tile([128, BTILES, 128], dt.float32, tag="sel")
                        nc.vector.tensor_tensor(
                            out=sel[:],
                            in0=iota_row[:, None, :].to_broadcast([128, BTILES, 128]),
                            in1=tsc[:].to_broadcast([128, BTILES, 128]),
                            op=OP.is_equal)
                        # --- x_i expansion + z
                        xjh = xj[:, half * BTILES:(half + 1) * BTILES, :]
                        zb = zp.tile([128, BTILES, HF], dt.float32)
                        for w8 in range(BWIN):
                            wp = half * BWIN + w8
                            psz = pz.tile([128, WTILES * HF], dt.float32,
                                          space="PSUM")
                            for j in range(WTILES):
                                t = w8 * WTILES + j
                                nc.tensor.matmul(
                                    out=psz[:, HF * j:HF * (j + 1)],
                                    lhsT=selT[:, 128 * t:128 * (t + 1)],
                                    rhs=hper[:, wp, :], start=True, stop=True)
                            nc.vector.tensor_tensor(
                                out=zb[:, w8 * WTILES:(w8 + 1) * WTILES, :],
                                in0=psz[:].rearrange("p (a f) -> p a f", f=HF),
                                in1=xjh[:, w8 * WTILES:(w8 + 1) * WTILES, :],
                                op=OP.add)
                        # --- leaky relu (ACT Prelu, exact)
                        nc.scalar.activation(out=zb[:], in_=zb[:], func=AF.Prelu,
                                             alpha=NEG_SLOPE)
                        # msgex: [...,:64] = msgs (first reused as z*att), [...,64:68] = exp
                        msgex = msgp.tile([128, BTILES, HF + H], dt.float32)
                        nc.vector.tensor_tensor(out=msgex[:, :, :HF], in0=zb[:],
                                                in1=att_t[:].rearrange(
                                                    "p (a f) -> p a f", f=HF),
                                                op=OP.mult)
                        sc = scp.tile([128, BTILES, H], dt.float32, tag="sc")
                        nc.vector.tensor_reduce(
                            out=sc[:],
                            in_=msgex[:, :, :HF].rearrange("p a (h f) -> p a h f", h=H),
                            axis=mybir.AxisListType.X, op=OP.add)
                        nc.scalar.activation(out=msgex[:, :, HF:], in_=sc[:],
                                             func=AF.Exp)
                        nc.vector.tensor_tensor(
                            out=msgex[:, :, :HF].rearrange("p a (h f) -> p a h f", h=H),
                            in0=xjh[:].rearrange("p a (h f) -> p a h f", h=H),
                            in1=msgex[:, :, HF:].to_broadcast([128, BTILES, H, F_OUT]),
                            op=OP.mult)
                        # --- aggregation into PSUM windows, then staging
                        for w8 in range(BWIN):
                            psw = pw.tile([128, HF + H], dt.float32, space="PSUM",
                                          tag="pswm")
                            for j in range(WTILES):
                                t = w8 * WTILES + j
                                nc.tensor.matmul(out=psw[:], lhsT=sel[:, t, :],
                                                 rhs=msgex[:, t, :],
                                                 start=(j == 0),
                                                 stop=(j == WTILES - 1))
                            nc.vector.tensor_copy(
                                out=stag[:, half * BWIN + w8, :HF + H], in_=psw[:])
                    nc.gpsimd.dma_scatter_add(accs[r][:], stag[:], sit[:],
                                              PW * 128, PW * 128, 128,
                                              single_packet=False, queue_num=qi % 4)
                    qi += 1

        # ---------------- phase M: merge + normalize + bias --------------
        with (
            tc.tile_pool(name="mp", bufs=3) as mp,
            tc.tile_pool(name="mo", bufs=3) as mo,
            tc.tile_pool(name="cst3", bufs=1) as cst3,
        ):
            if DEBUG:
                for r in range(NCHUNK):
                    for g in range(ACCROWS // 128):
                        at = mp.tile([128, 128], dt.float32, tag="dbgacc")
                        nc.sync.dma_start(
                            out=at[:], in_=accs[r][128 * g:128 * (g + 1), :])
                        nc.sync.dma_start(
                            out=dbg[f"acc{r}"][128 * g:128 * (g + 1), :],
                            in_=at[:])
            bt = cst3.tile([128, HF], dt.float32)
            nc.sync.dma_start(out=bt[:], in_=bias_b[:])
            for g in range(NLOCP // 128):
                a0 = mp.tile([128, 128], dt.float32, tag="a0")
                nc.sync.dma_start(out=a0[:], in_=accs[0][128 * g:128 * (g + 1), :])
                for r in range(1, NCHUNK):
                    ar = mp.tile([128, 128], dt.float32, tag=f"a{r}")
                    nc.sync.dma_start(out=ar[:], in_=accs[r][128 * g:128 * (g + 1), :])
                    nc.vector.tensor_tensor(out=a0[:], in0=a0[:], in1=ar[:],
                                            op=OP.add)
                den = mo.tile([128, H], dt.float32, tag="den")
                nc.vector.tensor_scalar(out=den[:], in0=a0[:, HF:HF + H],
                                        scalar1=1e-30, scalar2=None, op0=OP.max)
                rec = mo.tile([128, H], dt.float32, tag="rec")
                nc.vector.reciprocal(out=rec[:], in_=den[:])
                ot = mo.tile([128, HF], dt.float32, tag="ot")
                nc.vector.tensor_tensor(
                    out=ot[:].rearrange("p (h f) -> p h f", h=H),
                    in0=a0[:, :HF].rearrange("p (h f) -> p h f", h=H),
                    in1=rec[:].to_broadcast([128, H, F_OUT]), op=OP.mult)
                nc.vector.tensor_tensor(out=ot[:], in0=ot[:], in1=bt[:], op=OP.add)
                nc.sync.dma_start(out=out[128 * g:128 * (g + 1), :], in_=ot[:])

    nc.finalize()
    return nc


# ------------------------------------------------------------------- runner
_CACHE = {}


def _run(features, edge_index, weight_l, weight_r, att, bias, trace=False):
    from concourse.bass_utils import run_bass_kernel_spmd

    try:  # enable NTFF tracing under axon (missing antenv.axon_hooks shim)
        import antenv
        if "antenv.axon_hooks" not in sys.modules:
            from trn_agent_boot.trn_boot import _ntff_profile_via_ctypes
            hk = _ntff_profile_via_ctypes('/opt/axon/libaxon_pjrt.so')
            m = types.ModuleType("antenv.axon_hooks")
            m.get_axon_ntff_profile_hook = lambda: hk
            sys.modules["antenv.axon_hooks"] = m
            antenv.axon_hooks = m
    except Exception:
        pass

    features = np.asarray(features, dtype=np.float32)
    weight_l = np.asarray(weight_l, dtype=np.float32)
    weight_r = np.asarray(weight_r, dtype=np.float32)
    att = np.asarray(att, dtype=np.float32)
    bias = np.asarray(bias, dtype=np.float32)

    W_r, cores = prep(edge_index)
    key = tuple(W_r)
    if key not in _CACHE:
        _CACHE[key] = build(W_r)
    nc = _CACHE[key]

    featT = np.ascontiguousarray(features.T)                   # [128, N]
    w_lr = np.concatenate([weight_l, weight_r], axis=1)        # [128,128]
    att_row = att.reshape(1, HF).astype(np.float32)
    att_b = np.tile(att_row, (128, BTILES))                    # [128, 32*64]
    bias_b = np.tile(bias.reshape(1, HF), (128, 1))

    in_maps = []
    for c in range(NCORES):
        d = cores[c]
        n0 = c * NLOC
        floc = np.zeros((128, NLOCP), np.float32)
        hi = min(N, n0 + NLOCP)
        floc[:, :hi - n0] = featT[:, n0:hi]
        nt = d["tsel"].shape[0] // 128
        in_maps.append({
            "featT": featT, "featT_loc": floc, "w_lr": w_lr,
            "att_b": att_b, "bias_b": bias_b,
            "xj_idx": _wrap16(d["xj_idx"]),
            "tsel_rep": np.tile(d["tsel"][None, :], (128, 1)),
            "tsel_col": np.ascontiguousarray(
                d["tsel"].reshape(nt, 128).T).copy(),
            "pidx": _wrap16(d["pidx"]),
            "sidx": _wrap16(d["sidx"]),
            "acc0": np.zeros((ACCROWS, 128), np.float32),
            "acc1": np.zeros((ACCROWS, 128), np.float32),
            "acc2": np.zeros((ACCROWS, 128), np.float32),
            "acc3": np.zeros((ACCROWS, 128), np.float32),
        })

    res = run_bass_kernel_spmd(nc, in_maps, core_ids=list(range(NCORES)),
                               trace=trace)
    full = np.empty((N, HF), np.float32)
    for c in range(NCORES):
        full[c * NLOC:(c + 1) * NLOC] = res.results[c]["out"][:NLOC]
    return full, res


def kernel(features, edge_index, weight_l, weight_r, att, bias):
    out, _ = _run(features, edge_index, weight_l, weight_r, att, bias)
    return out



# revision 2
# speedup vs baseline: 2.2386x; 2.2386x over previous
"""GATv2 layer on 8 Trainium2 NeuronCores (Bass/Tile), v2.

Strategy (edge-parallel by target-node slice, no collectives, no scatters):
  - Node n belongs to core n // 12500. Targets grouped into 98 aligned blocks
    of 128 nodes; cells = (chunk r of 25088 src nodes, block b). Edges sorted
    (r, b), padded per-cell to 128-slot tiles (tile counts maxed over cores so
    all 8 cores run one SPMD program).
  - h_l for the core's slice and a [128, 98, 68] accumulator live in SBUF for
    the whole run -> no h_l gather, no scatter_add, no HBM merge phase.
  - h_r chunk tables ([25088, 64] f32, lane-permuted rows) are built on the
    fly; chunk r+1's table build is interleaved into chunk r's edge spans so
    PE/DMA never serialize against the gather stream.
  - Per (r, span of 8 blocks): one dma_gather fetches x_j rows (1 descriptor
    per slot, 256B, 4-queue rotation ~ 1.6ns/desc + B/137GB/s measured).
    x_i is expanded on-chip from SBUF h_l with one-hot selT matmuls (bf16);
    aggregation uses one-hot selq matmuls accumulating per-cell in PSUM, then
    a tiny DVE add into the SBUF accumulator.
  - Final: per block normalize (divide by exp-sum) + bias, DMA out.

Numerics: softmax without max-shift (scores O(+-12), safe in f32); messages,
sel matrices and h_l in bf16; scores in f32. rel err ~1e-3.
"""

import sys
import types

sys.path.insert(0, "/opt/trn_rl_repo")

import numpy as np

N, E, F_IN, H, F_OUT = 100000, 1600000, 128, 4, 16
HF = H * F_OUT            # 64
NEG_SLOPE = 0.2
NCORES = 8
NLOC = N // NCORES        # 12500
NLOCP = 12544             # 98*128
NB = NLOCP // 128         # 98 target blocks per core
CHUNK = 25088             # 196*128 src rows per chunk table (int16-safe rows)
NCHUNK = 4
NPAD = NCHUNK * CHUNK     # 100352 padded feature columns
SPB = 8                   # target blocks per span (gather batch)
NSPAN = -(-NB // SPB)     # 13
SB = 8                    # tiles per expansion sub-batch (psum 8*64 f32)


# ----------------------------------------------------------------- host prep
def _wrap16(ix):
    """int16 index layout for dma_gather: i -> (i%16, i//16), x8."""
    w2 = ix.reshape(-1, 16).T
    return np.tile(w2, (8, 1)).copy()


def prep(edge_index):
    src = np.asarray(edge_index[0], dtype=np.int64).astype(np.int32)
    tgt = np.asarray(edge_index[1], dtype=np.int64).astype(np.int32)
    percore = []
    cnts = np.zeros((NCORES, NCHUNK * NB), np.int64)
    for c in range(NCORES):
        n0 = c * NLOC
        m = (tgt >= n0) & (tgt < n0 + NLOC)
        s, t = src[m], tgt[m] - n0
        b = t >> 7
        r = s // CHUNK
        sloc = s - r * CHUNK
        tile = sloc >> 7
        # table row layout: store unit u holds tiles 4u..4u+3 lane-interleaved
        row = 512 * (tile >> 2) + 4 * (sloc & 127) + (tile & 3)
        cell = r * NB + b
        order = np.argsort(cell, kind="stable")
        cnts[c] = np.bincount(cell, minlength=NCHUNK * NB)
        percore.append((row[order].astype(np.int32),
                        (t & 127)[order].astype(np.int8), cnts[c]))
    T = (-(-cnts.max(axis=0) // 128)).astype(np.int64)   # tiles per cell
    cello = np.concatenate([[0], np.cumsum(T * 128)])
    S = int(cello[-1])
    outs = []
    for c in range(NCORES):
        row, tq, cellcnt = percore[c]
        xi = np.zeros(S, np.int16)
        ts = np.full(S, -1, np.int8)
        eo = np.concatenate([[0], np.cumsum(cellcnt)])
        cl = np.repeat(np.arange(NCHUNK * NB), cellcnt)
        pos = cello[cl] + (np.arange(len(row)) - eo[cl])
        xi[pos] = row
        ts[pos] = tq
        outs.append({
            "xj_idx": _wrap16(xi),
            "tsel_rep": np.ascontiguousarray(
                np.broadcast_to(ts[None, :], (128, S))),
            "tsel_col": np.ascontiguousarray(ts.reshape(-1, 128).T),
        })
    return tuple(T.tolist()), outs


# ------------------------------------------------------------- device kernel
def build(Tkey):
    import concourse.mybir as mybir
    import concourse.tile as tile
    from concourse import bacc

    dt = mybir.dt
    AF = mybir.ActivationFunctionType
    OP = mybir.AluOpType

    Tarr = np.asarray(Tkey, np.int64)                    # [NCHUNK*NB] r-major
    tile_off = np.concatenate([[0], np.cumsum(Tarr)])
    S = int(Tarr.sum()) * 128
    spans = []   # (r, t0, T_rg, cells=[(b, tcnt), ...])
    for r in range(NCHUNK):
        for g in range(NSPAN):
            b0, b1 = g * SPB, min((g + 1) * SPB, NB)
            cells = [(b, int(Tarr[r * NB + b])) for b in range(b0, b1)
                     if Tarr[r * NB + b] > 0]
            T_rg = sum(t for _, t in cells)
            if T_rg:
                spans.append((r, int(tile_off[r * NB + b0]), T_rg, cells))
    TM = max(sp[2] for sp in spans)

    nc = bacc.Bacc("TRN2", target_bir_lowering=False, num_swdge_queues=4,
                   dynamic_dma_scratch_size=32768)

    featT = nc.dram_tensor("featT", [128, NPAD], dt.float32,
                           kind="ExternalInput")
    featT_loc = nc.dram_tensor("featT_loc", [128, NLOCP], dt.float32,
                               kind="ExternalInput")
    w_lr = nc.dram_tensor("w_lr", [128, 2 * HF], dt.float32,
                          kind="ExternalInput")     # [:,:64]=w_l [:,64:]=w_r
    att_b = nc.dram_tensor("att_b", [128, HF], dt.float32,
                           kind="ExternalInput")
    bias_b = nc.dram_tensor("bias_b", [128, HF], dt.float32,
                            kind="ExternalInput")
    xj_idx = nc.dram_tensor("xj_idx", [128, S // 16], dt.int16,
                            kind="ExternalInput")
    tsel_rep = nc.dram_tensor("tsel_rep", [128, S], dt.int8,
                              kind="ExternalInput")
    tsel_col = nc.dram_tensor("tsel_col", [128, S // 128], dt.int8,
                              kind="ExternalInput")
    hr = [nc.dram_tensor(f"hr{r}", [CHUNK, HF], dt.float32, kind="Internal")
          for r in range(NCHUNK)]
    out = nc.dram_tensor("out", [NLOCP, HF], dt.float32, kind="ExternalOutput")

    qi = 0
    with tile.TileContext(nc) as tc:
        with (
            tc.tile_pool(name="cst", bufs=1) as cst,
            tc.tile_pool(name="hlp", bufs=1) as hlp,
            tc.tile_pool(name="accp", bufs=1) as accp,
            tc.tile_pool(name="hfp", bufs=3) as hfp,
            tc.tile_pool(name="hsp", bufs=3) as hsp,
            tc.tile_pool(name="hpp", bufs=2, space="PSUM") as hpp,
            tc.tile_pool(name="ixp", bufs=3) as ixp,
            tc.tile_pool(name="trp", bufs=3) as trp,
            tc.tile_pool(name="tcp", bufs=3) as tcp,
            tc.tile_pool(name="xjp", bufs=3) as xjp,
            tc.tile_pool(name="selp", bufs=2) as selp,
            tc.tile_pool(name="zp", bufs=2) as zp,
            tc.tile_pool(name="mp", bufs=2) as mp,
            tc.tile_pool(name="scp", bufs=2) as scp,
            tc.tile_pool(name="pz", bufs=2, space="PSUM") as pz,
            tc.tile_pool(name="pw", bufs=4, space="PSUM") as pw,
            tc.tile_pool(name="outp", bufs=3) as outp,
        ):
            # ---------------- constants ----------------------------------
            wt = cst.tile([128, 2 * HF], dt.float32)
            nc.sync.dma_start(out=wt[:], in_=w_lr[:])
            attf = cst.tile([128, HF], dt.float32)
            nc.sync.dma_start(out=attf[:], in_=att_b[:])
            att_t = cst.tile([128, HF], dt.bfloat16)
            nc.vector.tensor_copy(out=att_t[:], in_=attf[:])
            bias_t = cst.tile([128, HF], dt.float32)
            nc.sync.dma_start(out=bias_t[:], in_=bias_b[:])
            iota_i = cst.tile([128, 128], dt.int32)
            nc.gpsimd.iota(iota_i[:], pattern=[[1, 128]], base=0,
                           channel_multiplier=0)
            iota_row = cst.tile([128, 128], dt.int8)
            nc.vector.tensor_copy(out=iota_row[:], in_=iota_i[:])
            iota_c = cst.tile([128, 1], dt.int32)
            nc.gpsimd.iota(iota_c[:], pattern=[[0, 1]], base=0,
                           channel_multiplier=1)
            iota_col = cst.tile([128, 1], dt.float32)
            nc.vector.tensor_copy(out=iota_col[:], in_=iota_c[:])

            acc_t = accp.tile([128, NB * (HF + H)], dt.float32)
            nc.vector.memset(acc_t[:], 0.0)
            h_l_sb = hlp.tile([128, NB * HF], dt.bfloat16)

            # ---------------- phase H units ------------------------------
            def h_unit_table(r, u):
                ft = hfp.tile([128, 512], dt.float32, tag="ft")
                nc.scalar.dma_start(
                    out=ft[:],
                    in_=featT[:, r * CHUNK + 512 * u:r * CHUNK + 512 * (u + 1)])
                hp = hpp.tile([128, 256], dt.float32, space="PSUM")
                for j in range(4):
                    nc.tensor.matmul(out=hp[:, 64 * j:64 * (j + 1)],
                                     lhsT=ft[:, 128 * j:128 * (j + 1)],
                                     rhs=wt[:, HF:], start=True, stop=True)
                hs = hsp.tile([128, 256], dt.float32, tag="hs")
                nc.vector.tensor_copy(out=hs[:], in_=hp[:])
                nc.scalar.dma_start(out=hr[r][512 * u:512 * (u + 1), :],
                                    in_=hs[:])

            def h_unit_hl(v):
                c0 = 512 * v
                c1 = min(512 * (v + 1), NLOCP)
                w, ntl = c1 - c0, (c1 - c0) // 128
                fl = hfp.tile([128, 512], dt.float32, tag="ft")
                nc.scalar.dma_start(out=fl[:, :w], in_=featT_loc[:, c0:c1])
                hp = hpp.tile([128, 256], dt.float32, space="PSUM")
                for j in range(ntl):
                    nc.tensor.matmul(out=hp[:, 64 * j:64 * (j + 1)],
                                     lhsT=fl[:, 128 * j:128 * (j + 1)],
                                     rhs=wt[:, :HF], start=True, stop=True)
                nc.vector.tensor_copy(out=h_l_sb[:, c0 // 2:c0 // 2 + ntl * 64],
                                      in_=hp[:, :ntl * 64])

            for u in range(CHUNK // 512):         # table 0 first
                h_unit_table(0, u)
            for v in range(-(-NLOCP // 512)):     # h_l
                h_unit_hl(v)

            # ---------------- edge spans ---------------------------------
            def process(st):
                (r, t0, T_rg, cells), xj, selT, selq = st
                zb = zp.tile([128, TM, HF], dt.bfloat16, tag="zb")
                tb = [b for (b, tcnt) in cells for _ in range(tcnt)]
                done = 0
                while done < T_rg:
                    nsb = min(SB, T_rg - done)
                    psz = pz.tile([128, SB * HF], dt.float32, space="PSUM")
                    for j in range(nsb):
                        t = done + j
                        nc.tensor.matmul(
                            out=psz[:, HF * j:HF * (j + 1)],
                            lhsT=selT[:, 128 * t:128 * (t + 1)],
                            rhs=h_l_sb[:, tb[t] * HF:(tb[t] + 1) * HF],
                            start=True, stop=True)
                    nc.vector.tensor_tensor(
                        out=zb[:, done:done + nsb, :],
                        in0=psz[:, :nsb * HF].rearrange("p (t f) -> p t f",
                                                        f=HF),
                        in1=xj[:, done * HF:(done + nsb) * HF].rearrange(
                            "p (t f) -> p t f", f=HF),
                        op=OP.add)
                    done += nsb
                nc.scalar.activation(out=zb[:, :T_rg, :], in_=zb[:, :T_rg, :],
                                     func=AF.Prelu, alpha=NEG_SLOPE)
                nc.vector.tensor_tensor(
                    out=zb[:, :T_rg, :], in0=zb[:, :T_rg, :],
                    in1=att_t[:, None, :].to_broadcast([128, T_rg, HF]),
                    op=OP.mult)
                sc = scp.tile([128, TM, H], dt.float32, tag="sc")
                nc.vector.tensor_reduce(
                    out=sc[:, :T_rg, :],
                    in_=zb[:, :T_rg, :].rearrange("p t (h f) -> p t h f", h=H),
                    axis=mybir.AxisListType.X, op=OP.add)
                ex = scp.tile([128, TM, H], dt.float32, tag="ex")
                nc.scalar.activation(out=ex[:, :T_rg, :], in_=sc[:, :T_rg, :],
                                     func=AF.Exp)
                mx = mp.tile([128, TM, HF + H], dt.bfloat16, tag="mx")
                nc.vector.tensor_tensor(
                    out=mx[:, :T_rg, :HF].rearrange("p t (h f) -> p t h f",
                                                    h=H),
                    in0=xj[:, :T_rg * HF].rearrange("p (t h f) -> p t h f",
                                                    h=H, f=F_OUT),
                    in1=ex[:, :T_rg, :].to_broadcast([128, T_rg, H, F_OUT]),
                    op=OP.mult)
                nc.vector.tensor_copy(out=mx[:, :T_rg, HF:],
                                      in_=ex[:, :T_rg, :])
                t = 0
                for (b, tcnt) in cells:
                    psw = pw.tile([128, HF + H], dt.float32, space="PSUM")
                    for k in range(tcnt):
                        nc.tensor.matmul(out=psw[:], lhsT=selq[:, t + k, :],
                                         rhs=mx[:, t + k, :],
                                         start=(k == 0), stop=(k == tcnt - 1))
                    nc.vector.tensor_tensor(
                        out=acc_t[:, b * 68:b * 68 + 68],
                        in0=acc_t[:, b * 68:b * 68 + 68],
                        in1=psw[:], op=OP.add)
                    t += tcnt

            hq = {r: list(range(CHUNK // 512)) for r in range(1, NCHUNK)}
            per_span_h = -(-(CHUNK // 512) // NSPAN)
            pending = None
            for sp in spans:
                r, t0, T_rg, cells = sp
                n = 128 * T_rg
                ix = ixp.tile([128, TM * 8], dt.int16, tag="ix")
                nc.sync.dma_start(out=ix[:, :8 * T_rg],
                                  in_=xj_idx[:, 8 * t0:8 * (t0 + T_rg)])
                tr = trp.tile([128, TM * 128], dt.int8, tag="tr")
                nc.sync.dma_start(out=tr[:, :n],
                                  in_=tsel_rep[:, 128 * t0:128 * t0 + n])
                tcs = tcp.tile([128, TM], dt.int8, tag="tc")
                nc.sync.dma_start(out=tcs[:, :T_rg],
                                  in_=tsel_col[:, t0:t0 + T_rg])
                xj = xjp.tile([128, TM * HF], dt.float32, tag="xj")
                nc.gpsimd.dma_gather(
                    xj[:, :T_rg * HF].rearrange("p (t f) -> p t f", f=HF),
                    hr[r][:], ix[:, :8 * T_rg], n, n, HF,
                    single_packet=False, queue_num=qi % 4)
                qi += 1
                selT = selp.tile([128, TM * 128], dt.bfloat16, tag="selT")
                nc.vector.tensor_scalar(out=selT[:, :n], in0=tr[:, :n],
                                        scalar1=iota_col[:], scalar2=None,
                                        op0=OP.is_equal)
                selq = selp.tile([128, TM, 128], dt.bfloat16, tag="selq")
                nc.vector.tensor_tensor(
                    out=selq[:, :T_rg, :],
                    in0=iota_row[:, None, :].to_broadcast([128, T_rg, 128]),
                    in1=tcs[:, :T_rg].to_broadcast([128, T_rg, 128]),
                    op=OP.is_equal)
                if r + 1 < NCHUNK:
                    for _ in range(per_span_h):
                        if hq[r + 1]:
                            h_unit_table(r + 1, hq[r + 1].pop(0))
                if pending is not None:
                    process(pending)
                pending = (sp, xj, selT, selq)
            process(pending)

            # ---------------- finalize -----------------------------------
            for b in range(NB):
                den = outp.tile([128, H], dt.float32, tag="den")
                nc.vector.tensor_scalar(out=den[:],
                                        in0=acc_t[:, b * 68 + HF:b * 68 + 68],
                                        scalar1=1e-30, scalar2=None,
                                        op0=OP.max)
                rec = outp.tile([128, H], dt.float32, tag="rec")
                nc.vector.reciprocal(out=rec[:], in_=den[:])
                ot = outp.tile([128, HF], dt.float32, tag="ot")
                nc.vector.tensor_tensor(
                    out=ot[:].rearrange("p (h f) -> p h f", h=H),
                    in0=acc_t[:, b * 68:b * 68 + HF].rearrange(
                        "p (h f) -> p h f", h=H),
                    in1=rec[:].to_broadcast([128, H, F_OUT]), op=OP.mult)
                nc.vector.tensor_tensor(out=ot[:], in0=ot[:], in1=bias_t[:],
                                        op=OP.add)
                nc.sync.dma_start(out=out[128 * b:128 * (b + 1), :], in_=ot[:])

    nc.finalize()
    return nc


# ------------------------------------------------------------------- runner
_CACHE = {}


def _run(features, edge_index, weight_l, weight_r, att, bias, trace=False):
    from concourse.bass_utils import run_bass_kernel_spmd

    try:  # enable NTFF tracing under axon (missing antenv.axon_hooks shim)
        import antenv
        if "antenv.axon_hooks" not in sys.modules:
            from trn_agent_boot.trn_boot import _ntff_profile_via_ctypes
            hk = _ntff_profile_via_ctypes('/opt/axon/libaxon_pjrt.so')
            m = types.ModuleType("antenv.axon_hooks")
            m.get_axon_ntff_profile_hook = lambda: hk
            sys.modules["antenv.axon_hooks"] = m
            antenv.axon_hooks = m
    except Exception:
        pass

    features = np.asarray(features, dtype=np.float32)
    weight_l = np.asarray(weight_l, dtype=np.float32)
    weight_r = np.asarray(weight_r, dtype=np.float32)
    att = np.asarray(att, dtype=np.float32)
    bias = np.asarray(bias, dtype=np.float32)

    key, cores = prep(edge_index)
    if key not in _CACHE:
        _CACHE[key] = build(key)
    nc = _CACHE[key]

    featT = np.zeros((128, NPAD), np.float32)
    featT[:, :N] = np.ascontiguousarray(features.T)
    w_lrh = np.concatenate([weight_l, weight_r], axis=1)
    att_bh = np.tile(att.reshape(1, HF).astype(np.float32), (128, 1))
    bias_bh = np.tile(bias.reshape(1, HF), (128, 1))

    in_maps = []
    for c in range(NCORES):
        n0 = c * NLOC
        in_maps.append({
            "featT": featT,
            "featT_loc": np.ascontiguousarray(featT[:, n0:n0 + NLOCP]),
            "w_lr": w_lrh, "att_b": att_bh, "bias_b": bias_bh,
            **cores[c],
        })

    res = run_bass_kernel_spmd(nc, in_maps, core_ids=list(range(NCORES)),
                               trace=trace)
    full = np.empty((N, HF), np.float32)
    for c in range(NCORES):
        full[c * NLOC:(c + 1) * NLOC] = res.results[c]["out"][:NLOC]
    return full, res


def kernel(features, edge_index, weight_l, weight_r, att, bias):
    out, _ = _run(features, edge_index, weight_l, weight_r, att, bias)
    return out


# revision 7
# speedup vs baseline: 2.8503x; 1.2733x over previous
"""GATv2 layer on 8 Trainium2 NeuronCores (Bass/Tile), v2.

Strategy (edge-parallel by target-node slice, no collectives, no scatters):
  - Node n belongs to core n // 12500. Targets grouped into 98 aligned blocks
    of 128 nodes; cells = (chunk r of 25088 src nodes, block b). Edges sorted
    (r, b), padded per-cell to 128-slot tiles (tile counts maxed over cores so
    all 8 cores run one SPMD program).
  - h_l for the core's slice and a [128, 98, 68] accumulator live in SBUF for
    the whole run -> no h_l gather, no scatter_add, no HBM merge phase.
  - h_r chunk tables ([25088, 64] f32, lane-permuted rows) are built on the
    fly; chunk r+1's table build is interleaved into chunk r's edge spans so
    PE/DMA never serialize against the gather stream.
  - Per (r, span of 8 blocks): one dma_gather fetches x_j rows (1 descriptor
    per slot, 256B, 4-queue rotation ~ 1.6ns/desc + B/137GB/s measured).
    x_i is expanded on-chip from SBUF h_l with one-hot selT matmuls (bf16);
    aggregation uses one-hot selq matmuls accumulating per-cell in PSUM, then
    a tiny DVE add into the SBUF accumulator.
  - Final: per block normalize (divide by exp-sum) + bias, DMA out.

Numerics: softmax without max-shift (scores O(+-12), safe in f32); messages,
sel matrices and h_l in bf16; scores in f32. rel err ~1e-3.
"""

import sys
import types

sys.path.insert(0, "/opt/trn_rl_repo")

import numpy as np

N, E, F_IN, H, F_OUT = 100000, 1600000, 128, 4, 16
HF = H * F_OUT            # 64
NEG_SLOPE = 0.2
NCORES = 8
NLOC = N // NCORES        # 12500
NLOCP = 12544             # 98*128
NB = NLOCP // 128         # 98 target blocks per core
CHUNK = 25088             # 196*128 src rows per chunk table (int16-safe rows)
NCHUNK = 4
NPAD = NCHUNK * CHUNK     # 100352 padded feature columns
SPB = 8                   # target blocks per span (gather batch)
NSPAN = -(-NB // SPB)     # 13
SB = 8                    # tiles per expansion sub-batch (psum 8*64 f32)


# ----------------------------------------------------------------- host prep
def _wrap16(ix):
    """int16 index layout for dma_gather: i -> (i%16, i//16), x8."""
    w2 = ix.reshape(-1, 16).T
    return np.tile(w2, (8, 1)).copy()


def prep(edge_index):
    src = np.asarray(edge_index[0], dtype=np.int64).astype(np.int32)
    tgt = np.asarray(edge_index[1], dtype=np.int64).astype(np.int32)
    percore = []
    cnts = np.zeros((NCORES, NCHUNK * NB), np.int64)
    for c in range(NCORES):
        n0 = c * NLOC
        m = (tgt >= n0) & (tgt < n0 + NLOC)
        s, t = src[m], tgt[m] - n0
        b = t >> 7
        r = s // CHUNK
        sloc = s - r * CHUNK
        tile = sloc >> 7
        # table row layout: store unit u holds tiles 4u..4u+3 lane-interleaved
        row = 512 * (tile >> 2) + 4 * (sloc & 127) + (tile & 3)
        cell = r * NB + b
        order = np.argsort(cell, kind="stable")
        cnts[c] = np.bincount(cell, minlength=NCHUNK * NB)
        percore.append((row[order].astype(np.int32),
                        (t & 127)[order].astype(np.int8), cnts[c]))
    T = (-(-cnts.max(axis=0) // 128)).astype(np.int64)   # tiles per cell
    cello = np.concatenate([[0], np.cumsum(T * 128)])
    S = int(cello[-1])
    outs = []
    for c in range(NCORES):
        row, tq, cellcnt = percore[c]
        xi = np.zeros(S, np.int16)
        ts = np.full(S, -1, np.int8)
        eo = np.concatenate([[0], np.cumsum(cellcnt)])
        cl = np.repeat(np.arange(NCHUNK * NB), cellcnt)
        pos = cello[cl] + (np.arange(len(row)) - eo[cl])
        xi[pos] = row
        ts[pos] = tq
        outs.append({
            "xj_idx": _wrap16(xi),
            "tsel_rep": np.ascontiguousarray(
                np.broadcast_to(ts[None, :], (128, S))),
            "tsel_col": np.ascontiguousarray(ts.reshape(-1, 128).T),
        })
    return tuple(T.tolist()), outs


# ------------------------------------------------------------- device kernel
def build(Tkey):
    import concourse.mybir as mybir
    import concourse.tile as tile
    from concourse import bacc

    dt = mybir.dt
    AF = mybir.ActivationFunctionType
    OP = mybir.AluOpType

    Tarr = np.asarray(Tkey, np.int64)                    # [NCHUNK*NB] r-major
    tile_off = np.concatenate([[0], np.cumsum(Tarr)])
    S = int(Tarr.sum()) * 128
    spans = []   # (r, t0, T_rg, cells=[(b, tcnt), ...])
    for r in range(NCHUNK):
        for g in range(NSPAN):
            b0, b1 = g * SPB, min((g + 1) * SPB, NB)
            cells = [(b, int(Tarr[r * NB + b])) for b in range(b0, b1)
                     if Tarr[r * NB + b] > 0]
            T_rg = sum(t for _, t in cells)
            if T_rg:
                spans.append((r, int(tile_off[r * NB + b0]), T_rg, cells))
    TM = max(sp[2] for sp in spans)

    nc = bacc.Bacc("TRN2", target_bir_lowering=False, num_swdge_queues=4,
                   dynamic_dma_scratch_size=32768)

    featT = nc.dram_tensor("featT", [128, NPAD], dt.float32,
                           kind="ExternalInput")
    featT_loc = nc.dram_tensor("featT_loc", [128, NLOCP], dt.float32,
                               kind="ExternalInput")
    w_lr = nc.dram_tensor("w_lr", [128, 2 * HF], dt.float32,
                          kind="ExternalInput")     # [:,:64]=w_l [:,64:]=w_r
    att_b = nc.dram_tensor("att_b", [128, HF], dt.float32,
                           kind="ExternalInput")
    bias_b = nc.dram_tensor("bias_b", [128, HF], dt.float32,
                            kind="ExternalInput")
    xj_idx = nc.dram_tensor("xj_idx", [128, S // 16], dt.int16,
                            kind="ExternalInput")
    tsel_rep = nc.dram_tensor("tsel_rep", [128, S], dt.int8,
                              kind="ExternalInput")
    tsel_col = nc.dram_tensor("tsel_col", [128, S // 128], dt.int8,
                              kind="ExternalInput")
    hr = [nc.dram_tensor(f"hr{r}", [CHUNK, HF], dt.float32, kind="Internal")
          for r in range(NCHUNK)]
    out = nc.dram_tensor("out", [NLOCP, HF], dt.float32, kind="ExternalOutput")

    qi = 0
    from contextlib import ExitStack
    with tile.TileContext(nc) as tc:
        with ExitStack() as es:
            P = lambda *a, **k: es.enter_context(tc.tile_pool(*a, **k))
            cst = P(name="cst", bufs=1)
            hlp = P(name="hlp", bufs=1)
            accp = P(name="accp", bufs=1)
            hfp = P(name="hfp", bufs=2)
            hsp = P(name="hsp", bufs=2)
            hpp = P(name="hpp", bufs=2, space="PSUM")
            ixp = P(name="ixp", bufs=2)
            trp = P(name="trp", bufs=2)
            tcp = P(name="tcp", bufs=3)
            xjp = P(name="xjp", bufs=3)
            seltp = P(name="seltp", bufs=2)
            selqp = P(name="selqp", bufs=3)
            zpa = P(name="zpa", bufs=2)
            zpb = P(name="zpb", bufs=1)
            mp = P(name="mp", bufs=2)
            scp = P(name="scp", bufs=2)
            pz = P(name="pz", bufs=2, space="PSUM")
            pw = P(name="pw", bufs=4, space="PSUM")
            outp = P(name="outp", bufs=2)
            # ---------------- constants ----------------------------------
            wt = cst.tile([128, 2 * HF], dt.float32)
            nc.sync.dma_start(out=wt[:], in_=w_lr[:])
            attf = cst.tile([128, HF], dt.float32)
            nc.sync.dma_start(out=attf[:], in_=att_b[:])
            att_t = cst.tile([128, HF], dt.bfloat16)
            nc.vector.tensor_copy(out=att_t[:], in_=attf[:])
            bias_t = cst.tile([128, HF], dt.float32)
            nc.sync.dma_start(out=bias_t[:], in_=bias_b[:])
            iota_i = cst.tile([128, 128], dt.int32)
            nc.gpsimd.iota(iota_i[:], pattern=[[1, 128]], base=0,
                           channel_multiplier=0)
            iota_row = cst.tile([128, 128], dt.int8)
            nc.vector.tensor_copy(out=iota_row[:], in_=iota_i[:])
            iota_c = cst.tile([128, 1], dt.int32)
            nc.gpsimd.iota(iota_c[:], pattern=[[0, 1]], base=0,
                           channel_multiplier=1)
            iota_col = cst.tile([128, 1], dt.float32)
            nc.vector.tensor_copy(out=iota_col[:], in_=iota_c[:])
            iota_col8 = cst.tile([128, 1], dt.int8)
            nc.vector.tensor_copy(out=iota_col8[:], in_=iota_c[:])

            acc_t = accp.tile([128, NB * (HF + H)], dt.float32)
            nc.vector.memset(acc_t[:], 0.0)
            h_l_sb = hlp.tile([128, NB * HF], dt.bfloat16)

            # ---------------- phase H units ------------------------------
            def h_unit_table(r, u):
                ft = hfp.tile([128, 512], dt.float32, tag="ft")
                nc.scalar.dma_start(
                    out=ft[:],
                    in_=featT[:, r * CHUNK + 512 * u:r * CHUNK + 512 * (u + 1)])
                hp = hpp.tile([128, 256], dt.float32, space="PSUM")
                for j in range(4):
                    nc.tensor.matmul(out=hp[:, 64 * j:64 * (j + 1)],
                                     lhsT=ft[:, 128 * j:128 * (j + 1)],
                                     rhs=wt[:, HF:], start=True, stop=True)
                hs = hsp.tile([128, 256], dt.float32, tag="hs")
                nc.vector.tensor_copy(out=hs[:], in_=hp[:])
                nc.scalar.dma_start(out=hr[r][512 * u:512 * (u + 1), :],
                                    in_=hs[:])

            def h_unit_hl(v):
                c0 = 512 * v
                c1 = min(512 * (v + 1), NLOCP)
                w, ntl = c1 - c0, (c1 - c0) // 128
                fl = hfp.tile([128, 512], dt.float32, tag="ft")
                nc.scalar.dma_start(out=fl[:, :w], in_=featT_loc[:, c0:c1])
                hp = hpp.tile([128, 256], dt.float32, space="PSUM")
                for j in range(ntl):
                    nc.tensor.matmul(out=hp[:, 64 * j:64 * (j + 1)],
                                     lhsT=fl[:, 128 * j:128 * (j + 1)],
                                     rhs=wt[:, :HF], start=True, stop=True)
                nc.vector.tensor_copy(out=h_l_sb[:, c0 // 2:c0 // 2 + ntl * 64],
                                      in_=hp[:, :ntl * 64])

            for u in range(CHUNK // 512):         # table 0 first
                h_unit_table(0, u)
            for v in range(-(-NLOCP // 512)):     # h_l
                h_unit_hl(v)

            # ---------------- edge spans ---------------------------------
            def processA(st):
                """Expansion + z pipeline -> messages (mx)."""
                (r, t0, T_rg, cells), xj, selT, selq = st
                zb = zpa.tile([128, TM, HF], dt.float32, tag="zb")
                tb = [b for (b, tcnt) in cells for _ in range(tcnt)]
                done = 0
                while done < T_rg:
                    nsb = min(SB, T_rg - done)
                    psz = pz.tile([128, SB * HF], dt.float32, space="PSUM")
                    for j in range(nsb):
                        t = done + j
                        nc.tensor.matmul(
                            out=psz[:, HF * j:HF * (j + 1)],
                            lhsT=selT[:, 128 * t:128 * (t + 1)],
                            rhs=h_l_sb[:, tb[t] * HF:(tb[t] + 1) * HF],
                            start=True, stop=True)
                    nc.vector.tensor_tensor(
                        out=zb[:, done:done + nsb, :],
                        in0=psz[:, :nsb * HF].rearrange("p (t f) -> p t f",
                                                        f=HF),
                        in1=xj[:, done * HF:(done + nsb) * HF].rearrange(
                            "p (t f) -> p t f", f=HF),
                        op=OP.add)
                    done += nsb
                ub = zpb.tile([128, TM, HF], dt.bfloat16, tag="ub")
                nc.scalar.activation(out=ub[:, :T_rg, :], in_=zb[:, :T_rg, :],
                                     func=AF.Prelu, alpha=NEG_SLOPE)
                nc.vector.tensor_tensor(
                    out=ub[:, :T_rg, :], in0=ub[:, :T_rg, :],
                    in1=att_t[:, None, :].to_broadcast([128, T_rg, HF]),
                    op=OP.mult)
                sc = scp.tile([128, TM, H], dt.float32, tag="sc")
                nc.vector.tensor_reduce(
                    out=sc[:, :T_rg, :],
                    in_=ub[:, :T_rg, :].rearrange("p t (h f) -> p t h f", h=H),
                    axis=mybir.AxisListType.X, op=OP.add)
                mx = mp.tile([128, TM, HF + H], dt.bfloat16, tag="mx")
                nc.scalar.activation(out=mx[:, :T_rg, HF:],
                                     in_=sc[:, :T_rg, :], func=AF.Exp)
                nc.vector.tensor_tensor(
                    out=mx[:, :T_rg, :HF].rearrange("p t (h f) -> p t h f",
                                                    h=H),
                    in0=xj[:, :T_rg * HF].rearrange("p (t h f) -> p t h f",
                                                    h=H, f=F_OUT),
                    in1=mx[:, :T_rg, HF:].to_broadcast([128, T_rg, H, F_OUT]),
                    op=OP.mult)
                return mx

            def processB(st, mx):
                """Aggregation into per-cell PSUM, then SBUF accumulator."""
                (r, t0, T_rg, cells), xj, selT, selq = st
                t = 0
                for (b, tcnt) in cells:
                    psw = pw.tile([128, HF + H], dt.float32, space="PSUM")
                    for k in range(tcnt):
                        nc.tensor.matmul(out=psw[:], lhsT=selq[:, t + k, :],
                                         rhs=mx[:, t + k, :],
                                         start=(k == 0), stop=(k == tcnt - 1))
                    nc.vector.tensor_tensor(
                        out=acc_t[:, b * 68:b * 68 + 68],
                        in0=acc_t[:, b * 68:b * 68 + 68],
                        in1=psw[:], op=OP.add)
                    t += tcnt

            hq = {r: list(range(CHUNK // 512)) for r in range(1, NCHUNK)}
            per_span_h = -(-(CHUNK // 512) // NSPAN)
            stA = stB = None
            for sp in spans:
                r, t0, T_rg, cells = sp
                n = 128 * T_rg
                ix = ixp.tile([128, TM * 8], dt.int16, tag="ix")
                nc.sync.dma_start(out=ix[:, :8 * T_rg],
                                  in_=xj_idx[:, 8 * t0:8 * (t0 + T_rg)])
                tr = trp.tile([128, TM * 128], dt.int8, tag="tr")
                nc.sync.dma_start(out=tr[:, :n],
                                  in_=tsel_rep[:, 128 * t0:128 * t0 + n])
                tcs = tcp.tile([128, TM], dt.int8, tag="tc")
                nc.sync.dma_start(out=tcs[:, :T_rg],
                                  in_=tsel_col[:, t0:t0 + T_rg])
                xj = xjp.tile([128, TM * HF], dt.float32, tag="xj")
                nc.gpsimd.dma_gather(
                    xj[:, :T_rg * HF].rearrange("p (t f) -> p t f", f=HF),
                    hr[r][:], ix[:, :8 * T_rg], n, n, HF,
                    single_packet=False, queue_num=qi % 4)
                qi += 1
                selT = seltp.tile([128, TM * 128], dt.bfloat16, tag="selT")
                nc.vector.tensor_tensor(
                    out=selT[:, :n], in0=tr[:, :n],
                    in1=iota_col8[:].to_broadcast([128, n]),
                    op=OP.is_equal)
                selq = selqp.tile([128, TM, 128], dt.bfloat16, tag="selq")
                nc.vector.tensor_tensor(
                    out=selq[:, :T_rg, :],
                    in0=iota_row[:, None, :].to_broadcast([128, T_rg, 128]),
                    in1=tcs[:, :T_rg].to_broadcast([128, T_rg, 128]),
                    op=OP.is_equal)
                if r + 1 < NCHUNK:
                    for _ in range(per_span_h):
                        if hq[r + 1]:
                            h_unit_table(r + 1, hq[r + 1].pop(0))
                if stB is not None:
                    processB(*stB)
                if stA is not None:
                    stB = (stA, processA(stA))
                stA = (sp, xj, selT, selq)
            if stB is not None:
                processB(*stB)
            stB = (stA, processA(stA))
            processB(*stB)

            # ---------------- finalize (7-block groups) -------------------
            accv = acc_t[:].rearrange("p (b c) -> p b c", c=HF + H)
            for b0 in range(0, NB, 7):
                nb7 = min(7, NB - b0)
                den = outp.tile([128, 7, H], dt.float32, tag="den")
                nc.vector.tensor_scalar(out=den[:, :nb7, :],
                                        in0=accv[:, b0:b0 + nb7, HF:],
                                        scalar1=1e-30, scalar2=None,
                                        op0=OP.max)
                rec = outp.tile([128, 7, H], dt.float32, tag="rec")
                nc.vector.reciprocal(out=rec[:, :nb7, :], in_=den[:, :nb7, :])
                ot = outp.tile([128, 7, HF], dt.float32, tag="ot")
                nc.vector.tensor_tensor(
                    out=ot[:, :nb7, :].rearrange("p b (h f) -> p b h f", h=H),
                    in0=accv[:, b0:b0 + nb7, :HF].rearrange(
                        "p b (h f) -> p b h f", h=H),
                    in1=rec[:, :nb7, :].to_broadcast([128, nb7, H, F_OUT]),
                    op=OP.mult)
                nc.vector.tensor_tensor(
                    out=ot[:, :nb7, :], in0=ot[:, :nb7, :],
                    in1=bias_t[:, None, :].to_broadcast([128, nb7, HF]),
                    op=OP.add)
                for j in range(nb7):
                    b = b0 + j
                    nc.sync.dma_start(out=out[128 * b:128 * (b + 1), :],
                                      in_=ot[:, j, :])

    nc.finalize()
    return nc


# ------------------------------------------------------------------- runner
_CACHE = {}


def _run(features, edge_index, weight_l, weight_r, att, bias, trace=False):
    from concourse.bass_utils import run_bass_kernel_spmd

    try:  # enable NTFF tracing under axon (missing antenv.axon_hooks shim)
        import antenv
        if "antenv.axon_hooks" not in sys.modules:
            from trn_agent_boot.trn_boot import _ntff_profile_via_ctypes
            hk = _ntff_profile_via_ctypes('/opt/axon/libaxon_pjrt.so')
            m = types.ModuleType("antenv.axon_hooks")
            m.get_axon_ntff_profile_hook = lambda: hk
            sys.modules["antenv.axon_hooks"] = m
            antenv.axon_hooks = m
    except Exception:
        pass

    features = np.asarray(features, dtype=np.float32)
    weight_l = np.asarray(weight_l, dtype=np.float32)
    weight_r = np.asarray(weight_r, dtype=np.float32)
    att = np.asarray(att, dtype=np.float32)
    bias = np.asarray(bias, dtype=np.float32)

    key, cores = prep(edge_index)
    if key not in _CACHE:
        _CACHE[key] = build(key)
    nc = _CACHE[key]

    featT = np.zeros((128, NPAD), np.float32)
    featT[:, :N] = np.ascontiguousarray(features.T)
    w_lrh = np.concatenate([weight_l, weight_r], axis=1)
    att_bh = np.tile(att.reshape(1, HF).astype(np.float32), (128, 1))
    bias_bh = np.tile(bias.reshape(1, HF), (128, 1))

    in_maps = []
    for c in range(NCORES):
        n0 = c * NLOC
        in_maps.append({
            "featT": featT,
            "featT_loc": np.ascontiguousarray(featT[:, n0:n0 + NLOCP]),
            "w_lr": w_lrh, "att_b": att_bh, "bias_b": bias_bh,
            **cores[c],
        })

    res = run_bass_kernel_spmd(nc, in_maps, core_ids=list(range(NCORES)),
                               trace=trace)
    full = np.empty((N, HF), np.float32)
    for c in range(NCORES):
        full[c * NLOC:(c + 1) * NLOC] = res.results[c]["out"][:NLOC]
    return full, res


def kernel(features, edge_index, weight_l, weight_r, att, bias):
    out, _ = _run(features, edge_index, weight_l, weight_r, att, bias)
    return out


# revision 8
# speedup vs baseline: 3.0849x; 1.0823x over previous
"""GATv2 layer on 8 Trainium2 NeuronCores (Bass/Tile), v2.

Strategy (edge-parallel by target-node slice, no collectives, no scatters):
  - Node n belongs to core n // 12500. Targets grouped into 98 aligned blocks
    of 128 nodes; cells = (chunk r of 25088 src nodes, block b). Edges sorted
    (r, b), padded per-cell to 128-slot tiles (tile counts maxed over cores so
    all 8 cores run one SPMD program).
  - h_l for the core's slice and a [128, 98, 68] accumulator live in SBUF for
    the whole run -> no h_l gather, no scatter_add, no HBM merge phase.
  - h_r chunk tables ([25088, 64] f32, lane-permuted rows) are built on the
    fly; chunk r+1's table build is interleaved into chunk r's edge spans so
    PE/DMA never serialize against the gather stream.
  - Per (r, span of 8 blocks): one dma_gather fetches x_j rows (1 descriptor
    per slot, 256B, 4-queue rotation ~ 1.6ns/desc + B/137GB/s measured).
    x_i is expanded on-chip from SBUF h_l with one-hot selT matmuls (bf16);
    aggregation uses one-hot selq matmuls accumulating per-cell in PSUM, then
    a tiny DVE add into the SBUF accumulator.
  - Final: per block normalize (divide by exp-sum) + bias, DMA out.

Numerics: softmax without max-shift (scores O(+-12), safe in f32); messages,
sel matrices and h_l in bf16; scores in f32. rel err ~1e-3.
"""

import sys
import types

sys.path.insert(0, "/opt/trn_rl_repo")

import numpy as np

N, E, F_IN, H, F_OUT = 100000, 1600000, 128, 4, 16
HF = H * F_OUT            # 64
NEG_SLOPE = 0.2
NCORES = 8
NLOC = N // NCORES        # 12500
NLOCP = 12544             # 98*128
NB = NLOCP // 128         # 98 target blocks per core
CHUNK = 25088             # 196*128 src rows per chunk table (int16-safe rows)
NCHUNK = 4
NPAD = NCHUNK * CHUNK     # 100352 padded feature columns
SPB = 5                   # target blocks per span (gather batch)
NSPAN = -(-NB // SPB)     # 13
SB = 8                    # tiles per expansion sub-batch (psum 8*64 f32)


# ----------------------------------------------------------------- host prep
def _wrap16(ix):
    """int16 index layout for dma_gather: i -> (i%16, i//16), x8."""
    w2 = ix.reshape(-1, 16).T
    return np.tile(w2, (8, 1)).copy()


def prep(edge_index):
    src = np.asarray(edge_index[0], dtype=np.int64).astype(np.int32)
    tgt = np.asarray(edge_index[1], dtype=np.int64).astype(np.int32)
    percore = []
    cnts = np.zeros((NCORES, NCHUNK * NB), np.int64)
    for c in range(NCORES):
        n0 = c * NLOC
        m = (tgt >= n0) & (tgt < n0 + NLOC)
        s, t = src[m], tgt[m] - n0
        b = t >> 7
        r = s // CHUNK
        sloc = s - r * CHUNK
        tile = sloc >> 7
        # table row layout: store unit u holds tiles 4u..4u+3 lane-interleaved
        row = 512 * (tile >> 2) + 4 * (sloc & 127) + (tile & 3)
        cell = r * NB + b
        order = np.argsort(cell, kind="stable")
        cnts[c] = np.bincount(cell, minlength=NCHUNK * NB)
        percore.append((row[order].astype(np.int32),
                        (t & 127)[order].astype(np.int8), cnts[c]))
    T = (-(-cnts.max(axis=0) // 128)).astype(np.int64)   # tiles per cell
    cello = np.concatenate([[0], np.cumsum(T * 128)])
    S = int(cello[-1])
    outs = []
    for c in range(NCORES):
        row, tq, cellcnt = percore[c]
        xi = np.zeros(S, np.int16)
        ts = np.full(S, -1, np.int8)
        eo = np.concatenate([[0], np.cumsum(cellcnt)])
        cl = np.repeat(np.arange(NCHUNK * NB), cellcnt)
        pos = cello[cl] + (np.arange(len(row)) - eo[cl])
        xi[pos] = row
        ts[pos] = tq
        outs.append({
            "xj_idx": _wrap16(xi),
            "tsel_rep": np.ascontiguousarray(
                np.broadcast_to(ts[None, :], (128, S))),
            "tsel_col": np.ascontiguousarray(ts.reshape(-1, 128).T),
        })
    return tuple(T.tolist()), outs


# ------------------------------------------------------------- device kernel
def build(Tkey):
    import concourse.mybir as mybir
    import concourse.tile as tile
    from concourse import bacc

    dt = mybir.dt
    AF = mybir.ActivationFunctionType
    OP = mybir.AluOpType

    Tarr = np.asarray(Tkey, np.int64)                    # [NCHUNK*NB] r-major
    tile_off = np.concatenate([[0], np.cumsum(Tarr)])
    S = int(Tarr.sum()) * 128
    spans = []   # (r, t0, T_rg, cells=[(b, tcnt), ...])
    for r in range(NCHUNK):
        for g in range(NSPAN):
            b0, b1 = g * SPB, min((g + 1) * SPB, NB)
            cells = [(b, int(Tarr[r * NB + b])) for b in range(b0, b1)
                     if Tarr[r * NB + b] > 0]
            T_rg = sum(t for _, t in cells)
            if T_rg:
                spans.append((r, int(tile_off[r * NB + b0]), T_rg, cells))
    TM = max(sp[2] for sp in spans)

    nc = bacc.Bacc("TRN2", target_bir_lowering=False, num_swdge_queues=4,
                   dynamic_dma_scratch_size=32768)

    featT = nc.dram_tensor("featT", [128, NPAD], dt.float32,
                           kind="ExternalInput")
    featT_loc = nc.dram_tensor("featT_loc", [128, NLOCP], dt.float32,
                               kind="ExternalInput")
    w_lr = nc.dram_tensor("w_lr", [128, 2 * HF], dt.float32,
                          kind="ExternalInput")     # [:,:64]=w_l [:,64:]=w_r
    att_b = nc.dram_tensor("att_b", [128, HF], dt.float32,
                           kind="ExternalInput")
    bias_b = nc.dram_tensor("bias_b", [128, HF], dt.float32,
                            kind="ExternalInput")
    xj_idx = nc.dram_tensor("xj_idx", [128, S // 16], dt.int16,
                            kind="ExternalInput")
    tsel_rep = nc.dram_tensor("tsel_rep", [128, S], dt.int8,
                              kind="ExternalInput")
    tsel_col = nc.dram_tensor("tsel_col", [128, S // 128], dt.int8,
                              kind="ExternalInput")
    hr = [nc.dram_tensor(f"hr{r}", [CHUNK, HF], dt.float32, kind="Internal")
          for r in range(NCHUNK)]
    out = nc.dram_tensor("out", [NLOCP, HF], dt.float32, kind="ExternalOutput")

    qi = 0
    from contextlib import ExitStack
    with tile.TileContext(nc) as tc:
        with ExitStack() as es:
            P = lambda *a, **k: es.enter_context(tc.tile_pool(*a, **k))
            cst = P(name="cst", bufs=1)
            hlp = P(name="hlp", bufs=1)
            accp = P(name="accp", bufs=1)
            hfp = P(name="hfp", bufs=2)
            hsp = P(name="hsp", bufs=2)
            hpp = P(name="hpp", bufs=2, space="PSUM")
            ixp = P(name="ixp", bufs=3)
            trp = P(name="trp", bufs=3)
            tcp = P(name="tcp", bufs=3)
            xjp = P(name="xjp", bufs=4)
            seltp = P(name="seltp", bufs=3)
            selqp = P(name="selqp", bufs=4)
            zpa = P(name="zpa", bufs=2)
            zpb = P(name="zpb", bufs=2)
            mp = P(name="mp", bufs=3)
            scp = P(name="scp", bufs=2)
            pz = P(name="pz", bufs=2, space="PSUM")
            pw = P(name="pw", bufs=4, space="PSUM")
            outp = P(name="outp", bufs=2)
            # ---------------- constants ----------------------------------
            wt = cst.tile([128, 2 * HF], dt.float32)
            nc.sync.dma_start(out=wt[:], in_=w_lr[:])
            attf = cst.tile([128, HF], dt.float32)
            nc.sync.dma_start(out=attf[:], in_=att_b[:])
            att_t = cst.tile([128, HF], dt.bfloat16)
            nc.vector.tensor_copy(out=att_t[:], in_=attf[:])
            bias_t = cst.tile([128, HF], dt.float32)
            nc.sync.dma_start(out=bias_t[:], in_=bias_b[:])
            iota_i = cst.tile([128, 128], dt.int32)
            nc.gpsimd.iota(iota_i[:], pattern=[[1, 128]], base=0,
                           channel_multiplier=0)
            iota_row = cst.tile([128, 128], dt.int8)
            nc.vector.tensor_copy(out=iota_row[:], in_=iota_i[:])
            iota_c = cst.tile([128, 1], dt.int32)
            nc.gpsimd.iota(iota_c[:], pattern=[[0, 1]], base=0,
                           channel_multiplier=1)
            iota_col = cst.tile([128, 1], dt.float32)
            nc.vector.tensor_copy(out=iota_col[:], in_=iota_c[:])
            iota_col8 = cst.tile([128, 1], dt.int8)
            nc.vector.tensor_copy(out=iota_col8[:], in_=iota_c[:])

            acc_t = accp.tile([128, NB * (HF + H)], dt.float32)
            nc.vector.memset(acc_t[:], 0.0)
            h_l_sb = hlp.tile([128, NB * HF], dt.bfloat16)

            # ---------------- phase H units ------------------------------
            def h_unit_table(r, u):
                ft = hfp.tile([128, 512], dt.float32, tag="ft")
                nc.scalar.dma_start(
                    out=ft[:],
                    in_=featT[:, r * CHUNK + 512 * u:r * CHUNK + 512 * (u + 1)])
                hp = hpp.tile([128, 256], dt.float32, space="PSUM")
                for j in range(4):
                    nc.tensor.matmul(out=hp[:, 64 * j:64 * (j + 1)],
                                     lhsT=ft[:, 128 * j:128 * (j + 1)],
                                     rhs=wt[:, HF:], start=True, stop=True)
                hs = hsp.tile([128, 256], dt.float32, tag="hs")
                nc.vector.tensor_copy(out=hs[:], in_=hp[:])
                nc.scalar.dma_start(out=hr[r][512 * u:512 * (u + 1), :],
                                    in_=hs[:])

            def h_unit_hl(v):
                c0 = 512 * v
                c1 = min(512 * (v + 1), NLOCP)
                w, ntl = c1 - c0, (c1 - c0) // 128
                fl = hfp.tile([128, 512], dt.float32, tag="ft")
                nc.scalar.dma_start(out=fl[:, :w], in_=featT_loc[:, c0:c1])
                hp = hpp.tile([128, 256], dt.float32, space="PSUM")
                for j in range(ntl):
                    nc.tensor.matmul(out=hp[:, 64 * j:64 * (j + 1)],
                                     lhsT=fl[:, 128 * j:128 * (j + 1)],
                                     rhs=wt[:, :HF], start=True, stop=True)
                nc.vector.tensor_copy(out=h_l_sb[:, c0 // 2:c0 // 2 + ntl * 64],
                                      in_=hp[:, :ntl * 64])

            for u in range(CHUNK // 512):         # table 0 first
                h_unit_table(0, u)
            for v in range(-(-NLOCP // 512)):     # h_l
                h_unit_hl(v)

            # ---------------- edge spans ---------------------------------
            def processA(st):
                """Expansion + z pipeline -> messages (mx)."""
                (r, t0, T_rg, cells), xj, selT, selq = st
                zb = zpa.tile([128, TM, HF], dt.float32, tag="zb")
                tb = [b for (b, tcnt) in cells for _ in range(tcnt)]
                done = 0
                while done < T_rg:
                    nsb = min(SB, T_rg - done)
                    psz = pz.tile([128, SB * HF], dt.float32, space="PSUM")
                    for j in range(nsb):
                        t = done + j
                        nc.tensor.matmul(
                            out=psz[:, HF * j:HF * (j + 1)],
                            lhsT=selT[:, 128 * t:128 * (t + 1)],
                            rhs=h_l_sb[:, tb[t] * HF:(tb[t] + 1) * HF],
                            start=True, stop=True)
                    nc.vector.tensor_tensor(
                        out=zb[:, done:done + nsb, :],
                        in0=psz[:, :nsb * HF].rearrange("p (t f) -> p t f",
                                                        f=HF),
                        in1=xj[:, done * HF:(done + nsb) * HF].rearrange(
                            "p (t f) -> p t f", f=HF),
                        op=OP.add)
                    done += nsb
                ub = zpb.tile([128, TM, HF], dt.bfloat16, tag="ub")
                nc.scalar.activation(out=ub[:, :T_rg, :], in_=zb[:, :T_rg, :],
                                     func=AF.Prelu, alpha=NEG_SLOPE)
                nc.vector.tensor_tensor(
                    out=ub[:, :T_rg, :], in0=ub[:, :T_rg, :],
                    in1=att_t[:, None, :].to_broadcast([128, T_rg, HF]),
                    op=OP.mult)
                sc = scp.tile([128, TM, H], dt.float32, tag="sc")
                nc.vector.tensor_reduce(
                    out=sc[:, :T_rg, :],
                    in_=ub[:, :T_rg, :].rearrange("p t (h f) -> p t h f", h=H),
                    axis=mybir.AxisListType.X, op=OP.add)
                mx = mp.tile([128, TM, HF + H], dt.bfloat16, tag="mx")
                nc.scalar.activation(out=mx[:, :T_rg, HF:],
                                     in_=sc[:, :T_rg, :], func=AF.Exp)
                nc.vector.tensor_tensor(
                    out=mx[:, :T_rg, :HF].rearrange("p t (h f) -> p t h f",
                                                    h=H),
                    in0=xj[:, :T_rg * HF].rearrange("p (t h f) -> p t h f",
                                                    h=H, f=F_OUT),
                    in1=mx[:, :T_rg, HF:].to_broadcast([128, T_rg, H, F_OUT]),
                    op=OP.mult)
                return mx

            def processB(st, mx):
                """Aggregation into per-cell PSUM, then SBUF accumulator."""
                (r, t0, T_rg, cells), xj, selT, selq = st
                t = 0
                for (b, tcnt) in cells:
                    psw = pw.tile([128, HF + H], dt.float32, space="PSUM")
                    for k in range(tcnt):
                        nc.tensor.matmul(out=psw[:], lhsT=selq[:, t + k, :],
                                         rhs=mx[:, t + k, :],
                                         start=(k == 0), stop=(k == tcnt - 1))
                    nc.vector.tensor_tensor(
                        out=acc_t[:, b * 68:b * 68 + 68],
                        in0=acc_t[:, b * 68:b * 68 + 68],
                        in1=psw[:], op=OP.add)
                    t += tcnt

            hq = {r: list(range(CHUNK // 512)) for r in range(1, NCHUNK)}
            per_span_h = -(-(CHUNK // 512) // NSPAN)
            stA = stB = None
            for sp in spans:
                r, t0, T_rg, cells = sp
                n = 128 * T_rg
                ix = ixp.tile([128, TM * 8], dt.int16, tag="ix")
                nc.sync.dma_start(out=ix[:, :8 * T_rg],
                                  in_=xj_idx[:, 8 * t0:8 * (t0 + T_rg)])
                tr = trp.tile([128, TM * 128], dt.int8, tag="tr")
                nc.sync.dma_start(out=tr[:, :n],
                                  in_=tsel_rep[:, 128 * t0:128 * t0 + n])
                tcs = tcp.tile([128, TM], dt.int8, tag="tc")
                nc.sync.dma_start(out=tcs[:, :T_rg],
                                  in_=tsel_col[:, t0:t0 + T_rg])
                xj = xjp.tile([128, TM * HF], dt.float32, tag="xj")
                nc.gpsimd.dma_gather(
                    xj[:, :T_rg * HF].rearrange("p (t f) -> p t f", f=HF),
                    hr[r][:], ix[:, :8 * T_rg], n, n, HF,
                    single_packet=False, queue_num=qi % 4)
                qi += 1
                selT = seltp.tile([128, TM * 128], dt.bfloat16, tag="selT")
                nc.vector.tensor_tensor(
                    out=selT[:, :n], in0=tr[:, :n],
                    in1=iota_col8[:].to_broadcast([128, n]),
                    op=OP.is_equal)
                selq = selqp.tile([128, TM, 128], dt.bfloat16, tag="selq")
                nc.vector.tensor_tensor(
                    out=selq[:, :T_rg, :],
                    in0=iota_row[:, None, :].to_broadcast([128, T_rg, 128]),
                    in1=tcs[:, :T_rg].to_broadcast([128, T_rg, 128]),
                    op=OP.is_equal)
                if r + 1 < NCHUNK:
                    for _ in range(per_span_h):
                        if hq[r + 1]:
                            h_unit_table(r + 1, hq[r + 1].pop(0))
                if stB is not None:
                    processB(*stB)
                if stA is not None:
                    stB = (stA, processA(stA))
                stA = (sp, xj, selT, selq)
            if stB is not None:
                processB(*stB)
            stB = (stA, processA(stA))
            processB(*stB)

            # ---------------- finalize (7-block groups) -------------------
            accv = acc_t[:].rearrange("p (b c) -> p b c", c=HF + H)
            for b0 in range(0, NB, 7):
                nb7 = min(7, NB - b0)
                den = outp.tile([128, 7, H], dt.float32, tag="den")
                nc.vector.tensor_scalar(out=den[:, :nb7, :],
                                        in0=accv[:, b0:b0 + nb7, HF:],
                                        scalar1=1e-30, scalar2=None,
                                        op0=OP.max)
                rec = outp.tile([128, 7, H], dt.float32, tag="rec")
                nc.vector.reciprocal(out=rec[:, :nb7, :], in_=den[:, :nb7, :])
                ot = outp.tile([128, 7, HF], dt.float32, tag="ot")
                nc.vector.tensor_tensor(
                    out=ot[:, :nb7, :].rearrange("p b (h f) -> p b h f", h=H),
                    in0=accv[:, b0:b0 + nb7, :HF].rearrange(
                        "p b (h f) -> p b h f", h=H),
                    in1=rec[:, :nb7, :].to_broadcast([128, nb7, H, F_OUT]),
                    op=OP.mult)
                nc.vector.tensor_tensor(
                    out=ot[:, :nb7, :], in0=ot[:, :nb7, :],
                    in1=bias_t[:, None, :].to_broadcast([128, nb7, HF]),
                    op=OP.add)
                for j in range(nb7):
                    b = b0 + j
                    nc.sync.dma_start(out=out[128 * b:128 * (b + 1), :],
                                      in_=ot[:, j, :])

    nc.finalize()
    return nc


# ------------------------------------------------------------------- runner
_CACHE = {}


def _run(features, edge_index, weight_l, weight_r, att, bias, trace=False):
    from concourse.bass_utils import run_bass_kernel_spmd

    try:  # enable NTFF tracing under axon (missing antenv.axon_hooks shim)
        import antenv
        if "antenv.axon_hooks" not in sys.modules:
            from trn_agent_boot.trn_boot import _ntff_profile_via_ctypes
            hk = _ntff_profile_via_ctypes('/opt/axon/libaxon_pjrt.so')
            m = types.ModuleType("antenv.axon_hooks")
            m.get_axon_ntff_profile_hook = lambda: hk
            sys.modules["antenv.axon_hooks"] = m
            antenv.axon_hooks = m
    except Exception:
        pass

    features = np.asarray(features, dtype=np.float32)
    weight_l = np.asarray(weight_l, dtype=np.float32)
    weight_r = np.asarray(weight_r, dtype=np.float32)
    att = np.asarray(att, dtype=np.float32)
    bias = np.asarray(bias, dtype=np.float32)

    key, cores = prep(edge_index)
    if key not in _CACHE:
        _CACHE[key] = build(key)
    nc = _CACHE[key]

    featT = np.zeros((128, NPAD), np.float32)
    featT[:, :N] = np.ascontiguousarray(features.T)
    w_lrh = np.concatenate([weight_l, weight_r], axis=1)
    att_bh = np.tile(att.reshape(1, HF).astype(np.float32), (128, 1))
    bias_bh = np.tile(bias.reshape(1, HF), (128, 1))

    in_maps = []
    for c in range(NCORES):
        n0 = c * NLOC
        in_maps.append({
            "featT": featT,
            "featT_loc": np.ascontiguousarray(featT[:, n0:n0 + NLOCP]),
            "w_lr": w_lrh, "att_b": att_bh, "bias_b": bias_bh,
            **cores[c],
        })

    res = run_bass_kernel_spmd(nc, in_maps, core_ids=list(range(NCORES)),
                               trace=trace)
    full = np.empty((N, HF), np.float32)
    for c in range(NCORES):
        full[c * NLOC:(c + 1) * NLOC] = res.results[c]["out"][:NLOC]
    return full, res


def kernel(features, edge_index, weight_l, weight_r, att, bias):
    out, _ = _run(features, edge_index, weight_l, weight_r, att, bias)
    return out
